# revision 40
# baseline (speedup 1.0000x reference)
"""CLIPVisionEmbeddings Trainium2 kernel (LayerNorm folded through the matmul).

Computes, for full inputs:
    x   = LayerNorm(patches, g, b)                 # [N, PD]
    pe  = x @ W_patch.T                            # [N, D]
    pos = bbox_coords @ W_pos.T + b_pos            # [N, D]
    out = concat([cls_embed[0] + cls_pos, pe + pos])[None]   # [1, N+1, D]

Strategy: shard the N=32768 patch axis across 8 NeuronCores (4096 rows
each), replicate the small weights. The LayerNorm is folded through the
matmul so the PE consumes RAW x, pre-transposed AND pre-cast to bf16 on
the host (no on-device transposes, half the x DMA traffic):

    pe[n,:] = rstd_n*(x @ Wg.T) - rstd_n*mu_n*c1 + W@b
    (Wg = W*g, c1 = rowsum(Wg), W shipped as 32*Wg.T; the /32 rides the
     final per-row scale and keeps the optional fp8 mode out of subnormals)

Per core, the Tile kernel does, per 128-row tile:
  - LN stats via bn_stats/bn_aggr on a bf16 row-major copy of x (DVE)
  - std32 = 32*sqrt(var+eps) (ACT), rscale = 1/std32 (DVE)
  - 6 "aug" columns [bbox0..3*std32, std32, -32*mu] built on DVE; FOUR
    tiles' aug columns are packed (at col bases 32b) into ONE shared
    zero-padded [128,128] tile, PE-transposed once per quad, and the
    resulting full-[128,128] lhsT feeds each tile's rank-6 aug matmul
    against a per-tile MASKED copy of [W_pos.T; W@b+b_pos; c1] (rows
    32b..32b+5 of variant b, zeros elsewhere, built on-device) that
    accumulates positional + LN-correction terms into the same PSUM
    group (KERNEL_TPOSE=quad; -12us vs the 1-transpose-per-tile "pe"
    mode by cutting 24 transposes/iter and amortizing the aug lhsT
    across 8 matmuls)
  - 12 bf16 main matmuls (6 K-chunks x 2 PSUM halves) with the
    host-transposed x chunks as stationary lhsT
  - PSUM -> SBUF copy on ACT with per-partition scale rscale = rstd/32,
    bf16 output, DMA out issued from the idle Pool/gpsimd queue
  - the big transposed-x SBUF tile streams in via 8 chunked DMAs
    interleaved with the pipeline (the DMA fabric is the scarce resource
    at startup)

Measured dead ends (don't retry):
  - fp8 e4m3 DoubleRow mains (KERNEL_MM=fp8, 3-way hi/lo split): ~1.5x
    SLOWER than bf16 -- the 3 sweeps cost 18 DR matmuls vs 12 bf16 ones;
    DR only halves the K-dim cycles, so any >=2-sweep split loses. A
    1-sweep would be ~1.7x faster but its error (0.037 maxrel) fails the
    2e-2 gate; 2-sweep is 0.026 -- also fails. Structural, not a
    lowering bug.
  - Partial-tile PE ops are pathologically slow on this stack: K=6 or
    M=32 matmuls (incl. 4-way diagonal tile_position packing,
    KERNEL_TPOSE=dve) measured 3-5x total-kernel slowdowns. Always pad
    the stationary to full [128,128] (AUGPAD/PADT/quad modes do).
  - KERNEL_GTAIL=1 (bunched tail-group re-streams) and KERNEL_STAG=1
    measured slower than the default lazy per-group refill.
  - Stationary-reuse (W-stationary) isn't a lever: per-MM cost is
    ~237ns at N=512 whether the lhsT changes never, every 2nd, or every
    MM (LDWEIGHTS hides behind the 64-deep PE reorder window).
  - KERNEL_SQ=dve (std32/rscale via AluOpType.pow +-0.5, keeping the
    stats chain off ACT): passes CoreSim bit-identically but FAILS
    walrus codegen -- fractional pow doesn't lower.
  - KERNEL_CPENG=act (quad pt->augT copy on ACT) measured ~+12us
    (fights the drains); =pool (gpsimd) fails walrus codegen reading
    PSUM. DVE copy stays.
  - skip-aug ablation: the whole aug chain costs ~29us/iter (15us of
    aug-MM streams + ~12-14us of dependency stall) -- the stats->
    transpose->copy->aug-MM chain is the residual limiter if anyone
    wants the next 10us.

Host side only does layout/dtype transforms (transpose/cast/fp8-split)
plus small-weight prep: Wg, c1, crow, bbox transpose, cls row, gather.
"""

import os
import sys

import numpy as np

for _p in ("/opt/trn_rl_repo", "/root/.axon_site/_ro/trn_rl_repo"):
    if os.path.isdir(_p) and _p not in sys.path:
        sys.path.append(_p)

import ml_dtypes

N, PD, D = 32768, 768, 1024
NCORES = 8
RC = N // NCORES          # rows per core: 4096
NT = RC // 128            # n-tiles per core: 32
KT = PD // 128            # k-tiles: 6
EPS = 1e-5
WS = 32.0                 # host prescale on W (fp8 subnormal avoidance)

def _env():
    """Kernel config, re-read per build so one process can A/B variants:
    KERNEL_MM: main matmuls "fp8" (DoubleRow, 3-way hi/lo split) or "bf16"
    KERNEL_AUG: rank-6 aug matmul "fp8" (DoubleRow) or "bf16"
    KERNEL_YQ: y-DMA issue queue "pool" (gpsimd SWDGE) or "sp"
    KERNEL_AUGPAD: "1" pads the aug matmul to a full K=128 tile (bf16 aug only)
    """
    return (os.environ.get("KERNEL_MM", "bf16"),
            os.environ.get("KERNEL_AUG", "bf16"),
            os.environ.get("KERNEL_YQ", "pool"),
            os.environ.get("KERNEL_AUGPAD", "1") == "1",
            os.environ.get("KERNEL_PADT", "1") == "1",
            os.environ.get("KERNEL_TPOSE", "quad"),
            os.environ.get("KERNEL_CPENG", "dve"),
            os.environ.get("KERNEL_AUGENG", "dve"),
            os.environ.get("KERNEL_SQ", "act"))

_CACHE = {}


def _build_nc(repeat=None):
    import concourse.bass as bass
    import concourse.tile as tile
    from concourse import bacc, mybir
    from concourse.bass import ts
    from concourse.masks import make_identity

    f32 = mybir.dt.float32
    bf16 = mybir.dt.bfloat16
    f8 = mybir.dt.float8e4
    DR = mybir.MatmulPerfMode.DoubleRow

    def _b(name, default):
        return int(os.environ.get(f"KB_{name}", default))

    MM, AUG, YQ, AUGPAD, PADT, TPOSE, CPENG, AUGENG, SQ = _env()
    nc = bacc.Bacc("TRN2", target_bir_lowering=False, debug=False)

    x = nc.dram_tensor("x", [RC, PD], bf16, kind="ExternalInput").ap()
    if MM == "fp8":
        xh = nc.dram_tensor("xh", [128, KT, RC], f8, kind="ExternalInput").ap()
        xl = nc.dram_tensor("xl", [128, KT, RC], f8, kind="ExternalInput").ap()
        wh = nc.dram_tensor("wh", [128, KT, D], f8, kind="ExternalInput").ap()
        wl = nc.dram_tensor("wl", [128, KT, D], f8, kind="ExternalInput").ap()
    else:
        xb = nc.dram_tensor("xb", [128, KT, RC], bf16, kind="ExternalInput").ap()
        wb = nc.dram_tensor("wb", [128, KT, D], bf16, kind="ExternalInput").ap()
    bboxC = nc.dram_tensor("bboxC", [128, NT, 4], bf16, kind="ExternalInput").ap()
    # wposAug rows: 0-3 W_pos.T, 4 crow (= W_patch@ln1_b + b_pos), 5 c1 (= rowsum Wg)
    if AUG == "fp8":
        # DoubleRow layout [3, 2, D], rows scaled by 32 (augC columns carry /32)
        wposAug = nc.dram_tensor("wposAug", [3, 2, D], f8, kind="ExternalInput").ap()
    elif TPOSE == "dve":
        # replicated at partition bases 0/32/64/96 for the 4-way diagonal
        # tile_position aug matmuls (DVE stream-transpose scatters the aug
        # lhsT into those partition blocks)
        wposAug = nc.dram_tensor("wposAugR", [128, D], bf16, kind="ExternalInput").ap()
    else:
        wposAug = nc.dram_tensor("wposAug", [6, D], bf16, kind="ExternalInput").ap()
    y = nc.dram_tensor("y", [RC, D], bf16, kind="ExternalOutput").ap()

    from contextlib import ExitStack

    PF = _b("pf", 4 if TPOSE == "quad" else 2)  # stats pipeline depth
    NG = _b("ng", 8)  # xh/xl streaming DMA groups
    G = RC // NG
    TPG = NT // NG    # tiles per group

    with tile.TileContext(nc) as tc, ExitStack() as ctx:
        dve_t = TPOSE == "dve" and AUG != "fp8"
        quad_t = TPOSE == "quad" and AUG != "fp8"
        consts = ctx.enter_context(tc.tile_pool(name="consts", bufs=1))
        xhp = ctx.enter_context(tc.tile_pool(name="xhp", bufs=_b("xhp", 2)))
        xp = ctx.enter_context(tc.tile_pool(name="xp", bufs=_b("xp", 8)))
        sp = ctx.enter_context(tc.tile_pool(name="sp", bufs=_b("sp", 8)))
        ap_ = ctx.enter_context(tc.tile_pool(name="augp", bufs=_b("augp", 4)))
        tp = None
        if not dve_t:
            tp = ctx.enter_context(
                tc.tile_pool(name="tp", bufs=_b("tp", 2), space="PSUM"))
        mp = ctx.enter_context(
            tc.tile_pool(name="mp", bufs=_b("mp", 4 if dve_t else 3), space="PSUM"))
        op = ctx.enter_context(tc.tile_pool(name="op", bufs=_b("op", 4)))

        # --- constants ---
        if MM == "fp8":
            wh_sb = consts.tile([128, KT, D], f8)
            nc.sync.dma_start(out=wh_sb, in_=wh)
            wl_sb = consts.tile([128, KT, D], f8)
            nc.sync.dma_start(out=wl_sb, in_=wl)
        else:
            wb_sb = consts.tile([128, KT, D], bf16)
            nc.sync.dma_start(out=wb_sb, in_=wb)
        bboxC_sb = consts.tile([128, NT, 4], bf16)
        nc.sync.dma_start(out=bboxC_sb, in_=bboxC)
        augdt = f8 if AUG == "fp8" else bf16
        assert not (AUGPAD and AUG == "fp8"), "AUGPAD only implemented for bf16 aug"
        if dve_t:
            wposAug_sb = consts.tile([128, D], bf16)
            nc.sync.dma_start(out=wposAug_sb, in_=wposAug)
        elif quad_t:
            # masked variant b: the 6 aug rows at partitions 32b..32b+5 only,
            # so a shared 4-tile-packed [128,128] lhsT contracts just tile
            # (4q+b)'s rows; masks built on-device from the [6, D] input
            wposAug_sb = consts.tile([128, 4, D], bf16)
            nc.vector.memset(wposAug_sb, 0)
            for b in range(4):
                nc.sync.dma_start(out=wposAug_sb[32 * b:32 * b + 6, b, :],
                                  in_=wposAug)
        elif AUGPAD:
            wposAug_sb = consts.tile([128, D], augdt)
            nc.vector.memset(wposAug_sb, 0)
            nc.sync.dma_start(out=wposAug_sb[0:6, :], in_=wposAug)
        else:
            wposAug_sb = consts.tile([3, 2, D] if AUG == "fp8" else [6, D], augdt)
            nc.sync.dma_start(out=wposAug_sb, in_=wposAug)
        ident = None
        if not dve_t:
            ident = consts.tile([128, 128], bf16)
            make_identity(nc, ident)
        augCs = None
        if dve_t:
            # [128, 32] aug-column tiles for the DVE 32x32 stream transpose;
            # cols 6-31 stay zero from this one-time memset
            augCs = [sp.tile([128, 32], bf16, tag="augC", name=f"augC{j}")
                     for j in range(_b("sp", 8))]
            for t in augCs:
                nc.vector.memset(t, 0)
        elif quad_t:
            # one [128,128] transpose input per 4 tiles: tile i uses cols
            # 32*(i%4)..+6; rest stays zero from this one-time memset
            augCs = [sp.tile([128, 128], bf16, tag="augC", name=f"augC{j}")
                     for j in range(4)]
            for t in augCs:
                nc.vector.memset(t, 0)
        elif PADT:
            # full [128,128] transpose tiles (partial PE tiles are slow on
            # HW); cols 6-127 stay zero from this one-time memset
            augCs = [sp.tile([128, 128], bf16, tag="augC", name=f"augC{j}")
                     for j in range(_b("sp", 8))]
            for t in augCs:
                nc.vector.memset(t, 0)
        augTs = None
        if AUGPAD and not dve_t and not quad_t:
            augTs = [ap_.tile([128, 128], augdt, tag="augT", name=f"augT{j}")
                     for j in range(_b("augp", 4))]
            for t in augTs:
                nc.vector.memset(t, 0)
        eps_sb = consts.tile([128, 1], f32)
        nc.vector.memset(eps_sb, EPS * WS * WS)

        # timing-ablation switches; only honored in repeat/timing builds so
        # the graded single-shot build can never be affected
        skips = set()
        if repeat is not None:
            skips = set(os.environ.get("KERNEL_SKIP", "").split(","))

        def emit_stats(i):
            # row-major x tile -> LN stats -> scale/aug columns -> augT lhsT
            xt = xp.tile([128, PD], bf16, tag="xt")
            nc.sync.dma_start(out=xt, in_=x[ts(i, 128), :])

            stats = sp.tile([128, 2, 6], f32, tag="stats")
            xg = xt.rearrange("p (s f) -> p s f", s=2)
            for s in range(2):
                nc.vector.bn_stats(out=stats[:, s, :], in_=xg[:, s, :])
            mv = sp.tile([128, 2], f32, tag="mv")
            nc.vector.bn_aggr(out=mv, in_=stats)

            # std32 = sqrt(var*1024 + 1024*eps) = 32*sqrt(var+eps)
            aeng = nc.gpsimd if AUGENG == "pool" else nc.vector
            std32 = sp.tile([128, 1], f32, tag="std32")
            rscale = sp.tile([128, 1], f32, tag="rscale")
            if SQ == "dve":
                # keep the whole stats chain on one engine: no ACT round-trip
                # stalling the FIFO queue behind cross-engine waits
                v32 = sp.tile([128, 1], f32, tag="v32")
                aeng.tensor_scalar(out=v32, in0=mv[:, 1:2],
                                   scalar1=WS * WS, scalar2=WS * WS * EPS,
                                   op0=mybir.AluOpType.mult,
                                   op1=mybir.AluOpType.add)
                aeng.tensor_scalar(out=std32, in0=v32, scalar1=0.5,
                                   scalar2=None, op0=mybir.AluOpType.pow)
                aeng.tensor_scalar(out=rscale, in0=v32, scalar1=-0.5,
                                   scalar2=None, op0=mybir.AluOpType.pow)
            else:
                nc.scalar.activation(out=std32, in_=mv[:, 1:2],
                                     func=mybir.ActivationFunctionType.Sqrt,
                                     bias=eps_sb, scale=WS * WS)
                nc.vector.reciprocal(out=rscale, in_=std32)

            # aug columns (bbox0..3*s, s, -m) with (s, m) = (std32, 32*mu) for
            # the bf16 aug (unscaled wposAug) or (std, mu) for the fp8 aug
            # (wposAug rows pre-scaled by 32 on host; keeps e4m3 in range)
            cs = 1.0 if AUG != "fp8" else 1.0 / WS
            if AUG == "fp8":
                # k-tiles at columns 0:3 and 32:35 so the transposed psum can
                # be read at legal engine base partitions (0 and 32)
                augC = sp.tile([128, 64], bf16, tag="augC")
                nc.vector.tensor_scalar(out=augC[:, 0:3], in0=bboxC_sb[:, i, 0:3],
                                        scalar1=std32, scalar2=cs,
                                        op0=mybir.AluOpType.mult,
                                        op1=mybir.AluOpType.mult)
                nc.vector.tensor_scalar(out=augC[:, 32:33], in0=bboxC_sb[:, i, 3:4],
                                        scalar1=std32, scalar2=cs,
                                        op0=mybir.AluOpType.mult,
                                        op1=mybir.AluOpType.mult)
                nc.vector.tensor_scalar(out=augC[:, 33:34], in0=std32,
                                        scalar1=cs, scalar2=None,
                                        op0=mybir.AluOpType.mult)
                nc.vector.tensor_scalar(out=augC[:, 34:35], in0=mv[:, 0:1],
                                        scalar1=-WS * cs, scalar2=None,
                                        op0=mybir.AluOpType.mult)
                pt = tp.tile([64, 128], bf16, tag="pt")
                nc.tensor.transpose(out=pt, in_=augC, identity=ident)
                augT = ap_.tile([3, 2, 128], augdt, tag="augT")
                for k in range(2):
                    nc.vector.tensor_copy(out=augT[:, k, :],
                                          in_=pt[32 * k:32 * k + 3, :])
                return augT, rscale
            if quad_t:
                augC = augCs[(i // 4) % len(augCs)]
                c0 = 32 * (i % 4)
            elif PADT or dve_t:
                augC = augCs[i % len(augCs)]
                c0 = 0
            else:
                augC = sp.tile([128, 6], bf16, tag="augC")
                c0 = 0
            aeng.tensor_scalar(out=augC[:, c0:c0 + 4], in0=bboxC_sb[:, i, :],
                               scalar1=std32, scalar2=cs,
                               op0=mybir.AluOpType.mult,
                               op1=mybir.AluOpType.mult)
            aeng.tensor_scalar(out=augC[:, c0 + 4:c0 + 5], in0=std32,
                               scalar1=cs, scalar2=None,
                               op0=mybir.AluOpType.mult)
            aeng.tensor_scalar(out=augC[:, c0 + 5:c0 + 6], in0=mv[:, 0:1],
                               scalar1=-WS * cs, scalar2=None,
                               op0=mybir.AluOpType.mult)
            if quad_t:
                return augC, rscale

            if dve_t:
                # 32x32 block transpose on DVE: block b of augT4 holds the
                # aug rows for x-rows 32b..32b+31 at partitions 32b..32b+5
                augT4 = ap_.tile([128, 32], bf16, tag="augT")
                nc.vector.transpose(out=augT4, in_=augC)
                return augT4, rscale
            if PADT:
                pt = tp.tile([128, 128], bf16, tag="pt")
                nc.tensor.transpose(out=pt, in_=augC, identity=ident)
                augT = augTs[i % len(augTs)]
                nc.vector.tensor_copy(out=augT[0:6, :], in_=pt[0:6, :])
                return augT, rscale
            pt = tp.tile([6, 128], bf16, tag="pt")
            nc.tensor.transpose(out=pt, in_=augC, identity=ident)
            if AUGPAD:
                augT = augTs[i % len(augTs)]
                nc.vector.tensor_copy(out=augT[0:6, :], in_=pt)
            else:
                augT = ap_.tile([6, 128], augdt, tag="augT")
                nc.vector.tensor_copy(out=augT, in_=pt)
            return augT, rscale

        def emit_quad_tp(augC):
            # one padded full-tile transpose + copy covering 4 tiles' aug
            # columns; rows 32b+k of the result are tile (4q+b)'s aug row k
            pt = tp.tile([128, 128], bf16, tag="pt")
            nc.tensor.transpose(out=pt, in_=augC, identity=ident)
            augT = ap_.tile([128, 128], bf16, tag="augT")
            cpe = {"dve": nc.vector, "pool": nc.gpsimd, "act": nc.scalar}[CPENG]
            if CPENG == "act":
                cpe.activation(out=augT, in_=pt,
                               func=mybir.ActivationFunctionType.Copy)
            else:
                cpe.tensor_copy(out=augT, in_=pt)
            return augT

        def emit_mm(i, augT, rscale):
            ps0 = mp.tile([128, 512], f32, tag="ps0")
            ps1 = mp.tile([128, 512], f32, tag="ps1")
            pss = (ps0, ps1)
            if MM == "fp8":
                sets = [(xh_sb, wh_sb), (xl_sb, wh_sb), (xh_sb, wl_sb)]
                if "lo" in skips:  # timing-ablation only (reduced precision)
                    sets = sets[:1]
                for si, (xs, ws) in enumerate(sets):
                    for j in range(KT // 2):
                        for h in range(2):
                            nc.tensor.matmul(
                                pss[h],
                                lhsT=xs[:, 2 * j:2 * j + 2, ts(i, 128)],
                                rhs=ws[:, 2 * j:2 * j + 2, ts(h, 512)],
                                start=(si == 0 and j == 0), stop=False,
                                perf_mode=DR)
            else:
                for j in range(KT):
                    for h in range(2):
                        nc.tensor.matmul(
                            pss[h],
                            lhsT=xb_sb[:, j, ts(i, 128)],
                            rhs=wb_sb[:, j, ts(h, 512)],
                            start=(j == 0), stop=(dve_t and j == KT - 1))
            if "aug" not in skips:  # timing-ablation only (wrong results)
                for h in range(2):
                    if AUG == "fp8":
                        nc.tensor.matmul(pss[h], lhsT=augT,
                                         rhs=wposAug_sb[:, :, ts(h, 512)],
                                         start=False, stop=True, perf_mode=DR)
                    elif quad_t:
                        # shared full-[128,128] lhsT; the masked rhs variant
                        # b selects this tile's 6 aug rows
                        nc.tensor.matmul(pss[h], lhsT=augT,
                                         rhs=wposAug_sb[:, i % 4, ts(h, 512)],
                                         start=False, stop=True)
                    elif dve_t:
                        # 4 diagonal-block K=6 matmuls (row_grp b, col_grp b)
                        # run concurrently in the PE array
                        for b in range(4):
                            p = 32 * b
                            nc.tensor.matmul(
                                pss[h][p:p + 32, :],
                                lhsT=augT[p:p + 6, 0:32],
                                rhs=wposAug_sb[p:p + 6, ts(h, 512)],
                                start=False, stop=False,
                                skip_group_check=True,
                                tile_position=(p, p))
                    else:
                        nc.tensor.matmul(pss[h], lhsT=augT,
                                         rhs=wposAug_sb[:, ts(h, 512)],
                                         start=False, stop=True)

            ot = op.tile([128, D], bf16)
            for h in range(2):
                nc.scalar.activation(out=ot[:, ts(h, 512)], in_=pss[h],
                                     func=mybir.ActivationFunctionType.Copy,
                                     scale=rscale)
            # issue from the (idle) Pool SWDGE queue: y waits on the ACT
            # copies, and a waiting DMA blocks its whole issue queue -- on
            # SP that would head-of-line block the x/xh/xl input stream
            if "ydma" not in skips:  # timing-ablation only (no output)
                yq = nc.gpsimd if YQ == "pool" else nc.sync
                yq.dma_start(out=y[ts(i, 128), :], in_=ot)

        def emit_group(g):
            # stream group g of the transposed fp8 x into SBUF; lazy emission
            # keeps the (serialized) DMA fabric free for the per-tile x DMAs
            # that feed the latency-critical stats/aug path
            if MM == "fp8":
                nc.sync.dma_start(out=xh_sb[:, :, ts(g, G)],
                                  in_=xh[:, :, ts(g, G)])
                nc.sync.dma_start(out=xl_sb[:, :, ts(g, G)],
                                  in_=xl[:, :, ts(g, G)])
            else:
                nc.sync.dma_start(out=xb_sb[:, :, ts(g, G)],
                                  in_=xb[:, :, ts(g, G)])

        def alloc_x():
            nonlocal xh_sb, xl_sb, xb_sb
            if MM == "fp8":
                xh_sb = xhp.tile([128, KT, RC], f8, tag="xh")
                xl_sb = xhp.tile([128, KT, RC], f8, tag="xl")
            else:
                xb_sb = xhp.tile([128, KT, RC], bf16, tag="xb")

        def body(rep, tail_groups=False):
            # software pipeline: stats/aug path runs PF tiles ahead of the
            # matmuls so the aug lhsT is always ready when the mains finish
            pend = []
            next_g = 1
            stats0 = None
            qTs = {}

            def pop_mm():
                j, a, r = pend.pop(0)
                if quad_t:
                    a = qTs[j // 4]
                emit_mm(j, a, r)

            for i in range(NT):
                if "stats" in skips:  # timing-ablation only (wrong results)
                    if stats0 is None:
                        stats0 = emit_stats(i)
                    pend.append((i, *stats0))
                else:
                    pend.append((i, *emit_stats(i)))
                if quad_t and i % 4 == 3:
                    qTs[i // 4] = emit_quad_tp(pend[-1][1])
                if tail_groups:
                    # repeat/timing mode: x was preloaded outside the loop;
                    # re-stream each group late in the iteration (its readers
                    # are long done) so the next iteration starts with data
                    # already resident -- removes the back-edge refill stall
                    g = i - (NT - NG - 2)
                    if 0 <= g < NG:
                        emit_group(g)
                else:
                    # single-shot: lazy-emit group g just before its first
                    # reader so the DMA fabric stays free for the stats path
                    if next_g < NG and i >= next_g * TPG - 3:
                        emit_group(next_g)
                        next_g += 1
                if len(pend) > PF:
                    pop_mm()
            while pend:
                pop_mm()

        xh_sb = xl_sb = xb_sb = None
        if repeat is None:
            alloc_x()
            emit_group(0)
            body(0)
        else:
            alloc_x()
            for g in range(NG):
                emit_group(g)
            hints = (mybir.EngineType.PE, mybir.EngineType.DVE,
                     mybir.EngineType.Activation, mybir.EngineType.SP,
                     mybir.EngineType.Pool)
            if os.environ.get("KERNEL_NOPOOLHINT", "0") == "1":
                hints = hints[:4]
            stag = os.environ.get("KERNEL_STAG", "0") == "1"
            with tc.For_i(0, repeat, 1, hint_engines=hints,
                          staggered_reset=stag):
                body(0, tail_groups=os.environ.get("KERNEL_GTAIL", "0") == "1")

    nc.compile()
    return nc


def _host_prep(patches, bbox_coords, ln1_g, ln1_b, W_patch, cls_embed,
               W_pos, b_pos, cls_pos):
    MM, AUG, YQ, AUGPAD, _PADT, TPOSE, _CPENG, _AUGENG, _SQ = _env()
    f8 = ml_dtypes.float8_e4m3
    bf16 = ml_dtypes.bfloat16

    Wg = W_patch.astype(np.float32) * ln1_g.astype(np.float32)[None, :]  # [D, PD]
    W32T = np.ascontiguousarray(Wg.T) * np.float32(WS)                   # [PD, D]
    # pack [PD, D] -> [128, KT, D]
    w_pack = W32T.reshape(KT, 128, D).transpose(1, 0, 2)
    if MM == "fp8":
        wh = np.ascontiguousarray(w_pack).astype(f8)
        wl = (w_pack - wh.astype(np.float32)).astype(f8)
        wl = np.ascontiguousarray(wl)
    else:
        wb = np.ascontiguousarray(w_pack).astype(bf16)

    crow = (W_patch.astype(np.float64) @ ln1_b.astype(np.float64)
            + b_pos.astype(np.float64)).astype(np.float32)               # [D]
    c1 = Wg.sum(axis=1, dtype=np.float64).astype(np.float32)             # [D]
    wposAug = np.empty((6, D), np.float32)
    wposAug[0:4] = W_pos.astype(np.float32).T
    wposAug[4] = crow
    wposAug[5] = c1
    wpos_key = "wposAug"
    if AUG == "fp8":
        # [row, D] -> [3, 2, D] DoubleRow layout: [p, k, :] = row 3k+p
        wposAug = np.ascontiguousarray(
            (wposAug * np.float32(WS)).reshape(2, 3, D).transpose(1, 0, 2)).astype(f8)
    elif TPOSE == "dve":
        # replicate the 6 rows at partition bases 0/32/64/96
        wpos_key = "wposAugR"
        rep = np.zeros((128, D), np.float32)
        for b in range(4):
            rep[32 * b:32 * b + 6] = wposAug
        wposAug = rep.astype(bf16)
    else:
        wposAug = wposAug.astype(bf16)

    cls_row = (cls_embed[0, 0].astype(np.float32)
               + cls_pos[0].astype(np.float32))                          # [D]

    patches = patches.astype(np.float32)
    in_maps = []
    for c in range(NCORES):
        sl = slice(c * RC, (c + 1) * RC)
        xc = patches[sl]                                                 # [RC, PD]
        # transposed pack [PD, RC] -> [128, KT, RC]
        xT = np.ascontiguousarray(xc.T).reshape(KT, 128, RC).transpose(1, 0, 2)
        m = {
            "x": np.ascontiguousarray(xc).astype(bf16),
            "bboxC": np.ascontiguousarray(
                bbox_coords[sl].astype(np.float32).reshape(NT, 128, 4)
                .transpose(1, 0, 2)).astype(bf16),
            wpos_key: wposAug,
        }
        if MM == "fp8":
            xh = np.ascontiguousarray(xT).astype(f8)
            xlv = (xT - xh.astype(np.float32)).astype(f8)
            m["xh"] = xh
            m["xl"] = np.ascontiguousarray(xlv)
            m["wh"] = wh
            m["wl"] = wl
        else:
            m["xb"] = np.ascontiguousarray(xT).astype(bf16)
            m["wb"] = wb
        in_maps.append(m)
    return in_maps, cls_row


def get_nc(repeat=None):
    key = ("nc", *_env(), repeat)
    if key not in _CACHE:
        _CACHE[key] = _build_nc(repeat)
    return _CACHE[key]


def kernel(**inputs):
    from concourse import bass_utils

    inputs = {k: np.asarray(v) for k, v in inputs.items()}
    in_maps, cls_row = _host_prep(**inputs)
    nc = get_nc()
    res = bass_utils.run_bass_kernel_spmd(nc, in_maps, core_ids=list(range(NCORES)))
    out = np.empty((1, N + 1, D), np.float32)
    out[0, 0] = cls_row
    for c in range(NCORES):
        out[0, 1 + c * RC: 1 + (c + 1) * RC] = res.results[c]["y"].astype(np.float32)
    return out

